# revision 1
# baseline (speedup 1.0000x reference)
"""ErnieLayout self-attention Trainium2 kernel.

Shards batch x heads over 8 NeuronCores: cores 0-3 take batch 0, cores 4-7
take batch 1, 3 heads each (data-parallel on batch, tensor-parallel on heads).
No cross-core communication; host slices inputs and gathers the per-core
[S, 192] outputs.

Per-core pipeline (all matmuls fp16 with fp32 PSUM accumulation; softmax
math in fp32; rel err vs fp32 reference ~2e-3):
  - one-time: hs and the weight slices are transposed on the PE (as regular
    identity matmuls); q|k packed projections produce qT/kT [64, S] with the
    1/sqrt(64) scale and biases folded in (biases enter as a K=1 matmul
    against a ones row); v_aug [S, 65] carries a ones column so the PV
    matmul also emits softmax denominators.
  - per (s-tile, head): rel_pos arrives as an fp32->fp16 SWDGE cast-DMA
    (rel_2d_pos accumulates onto it in-DMA for 1/8 of tiles, else it is a
    second cast-DMA added on the PE); scores = rel(+rel2d) + q k^T all
    accumulate in PSUM via identity matmuls; DVE evicts PSUM with a fused
    +mask*(-30000) add -> fp16 logits; logit 128x128 blocks are transposed
    with regular identity matmuls into PSUM; ACT exp's PSUM->SBUF (fused
    eviction) -> probsT; PV matmul contracts t on the partition axis giving
    ctx plus row sums; DVE reciprocal-multiplies and one contiguous DMA per
    s-tile writes [128, 192] out.
"""

import numpy as np

B, S, HID = 2, 2048, 768
NH, HD = 12, 64
NCORES = 8
HPC = 3          # heads per core
NEG = -30000.0   # additive mask constant; exp(-30000) == 0.0 in fp32

_CACHE = {}


def _build():
    from contextlib import ExitStack

    import concourse.bacc as bacc
    import concourse.tile as tile
    from concourse import mybir
    from concourse.masks import make_identity

    fp32 = mybir.dt.float32
    bf16 = mybir.dt.float16  # fp16: same speed, 8x mantissa of bf16; all values < 65504
    i8 = mybir.dt.int8
    Alu = mybir.AluOpType

    nc = bacc.Bacc(
        "TRN2",
        target_bir_lowering=False,
        debug=False,
        enable_asserts=False,
        num_devices=NCORES,
    )

    hs_d = nc.dram_tensor("hs", (S, HID), fp32, kind="ExternalInput").ap()
    wq_d = nc.dram_tensor("wq", (HPC * HD, HID), fp32, kind="ExternalInput").ap()
    wk_d = nc.dram_tensor("wk", (HPC * HD, HID), fp32, kind="ExternalInput").ap()
    wv_d = nc.dram_tensor("wv", (HPC * HD, HID), fp32, kind="ExternalInput").ap()
    bq_d = nc.dram_tensor("bq", (1, HPC * HD), fp32, kind="ExternalInput").ap()
    bk_d = nc.dram_tensor("bk", (1, HPC * HD), fp32, kind="ExternalInput").ap()
    bv_d = nc.dram_tensor("bv", (1, HPC * HD), fp32, kind="ExternalInput").ap()
    rel_d = nc.dram_tensor("rel", (HPC, S, S), fp32, kind="ExternalInput").ap()
    rel2_d = nc.dram_tensor("rel2", (HPC, S, S), fp32, kind="ExternalInput").ap()
    mask_d = nc.dram_tensor("mask", (S, S), i8, kind="ExternalInput").ap()
    out_d = nc.dram_tensor("out", (S, HPC * HD), fp32, kind="ExternalOutput").ap()

    NWID = HPC * HD    # 192
    NSC = S // 128     # 16 s-tiles
    NKC = HID // 128   # 6 contraction chunks

    with tile.TileContext(nc) as tc, ExitStack() as top:
        persist = top.enter_context(tc.tile_pool(name="persist", bufs=1))

        ident = persist.tile([128, 128], bf16, tag="ident")
        make_identity(nc, ident)
        ones_row = persist.tile([1, S], bf16, tag="ones_row")
        nc.vector.memset(ones_row, 1.0)

        # hsT[:, kc, s] = hs[s, kc*128 + p] as fp16
        hsT = persist.tile([128, NKC, S], bf16, tag="hsT")
        # packed q|k weights: wqkT[:, kc, h*128 + 0:64] = WqT head h,
        #                     wqkT[:, kc, h*128 + 64:128] = WkT head h
        wqkT = persist.tile([128, NKC, HPC * 128], bf16, tag="wqkT")
        bias_qk = persist.tile([1, HPC * 128], bf16, tag="bias_qk")
        # v weights kept unpacked
        wvT = persist.tile([128, NKC, NWID], bf16, tag="wvT")
        bias_v = persist.tile([1, NWID], bf16, tag="bias_v")

        # ---- Phases 0+1 interleaved: weights first, then per 512-col
        # group: 4 hs tiles -> hsT -> q/k projections (all heads) -> v.
        # Attention h=0 can start right after the last group instead of
        # waiting for a fully serial load->transpose->project pipeline.
        qT = [persist.tile([64, S], bf16, tag=f"qT{h}", name=f"qT{h}") for h in range(HPC)]
        kT = [persist.tile([64, S], bf16, tag=f"kT{h}", name=f"kT{h}") for h in range(HPC)]
        v_aug = [
            persist.tile([128, NSC, HD + 1], bf16, tag=f"vaug{h}", name=f"vaug{h}")
            for h in range(HPC)
        ]
        for h in range(HPC):
            nc.vector.memset(v_aug[h], 1.0)

        with ExitStack() as ph:
            stage = ph.enter_context(tc.tile_pool(name="stage", bufs=4))
            tps = ph.enter_context(tc.tile_pool(name="tps", bufs=3, space="PSUM"))
            pps = ph.enter_context(tc.tile_pool(name="pps", bufs=3, space="PSUM"))

            for w, (w_d, b_d) in enumerate(((wq_d, bq_d), (wk_d, bk_d))):
                btmp = stage.tile([1, NWID], fp32, tag="btmp")
                nc.sync.dma_start(out=btmp, in_=b_d)
                btmp_bf = stage.tile([1, NWID], bf16, tag="btmp_bf")
                nc.vector.tensor_copy(btmp_bf, btmp)
                for h in range(HPC):
                    nc.vector.tensor_copy(
                        bias_qk[:, h * 128 + w * HD : h * 128 + (w + 1) * HD],
                        btmp_bf[:, h * HD : (h + 1) * HD],
                    )
                    wrow_bf = stage.tile([HD, HID], bf16, tag="wrow_bf")
                    nc.gpsimd.dma_start(
                        out=wrow_bf, in_=w_d[h * HD : (h + 1) * HD, :]
                    )
                    for kc in range(NKC):
                        tp = tps.tile([128, HD], fp32, tag="tpw")
                        nc.tensor.matmul(
                            tp,
                            lhsT=wrow_bf[:, kc * 128 : (kc + 1) * 128],
                            rhs=ident[:HD, :HD],
                        )
                        nc.scalar.copy(
                            wqkT[:, kc, h * 128 + w * HD : h * 128 + (w + 1) * HD], tp
                        )

            btmp2 = stage.tile([1, NWID], fp32, tag="btmp")
            nc.sync.dma_start(out=btmp2, in_=bv_d)
            nc.vector.tensor_copy(bias_v, btmp2)
            for rc in range(2):
                wrow_bf2 = stage.tile([96, HID], bf16, tag="wrow_bf2")
                nc.gpsimd.dma_start(out=wrow_bf2, in_=wv_d[rc * 96 : (rc + 1) * 96, :])
                for kc in range(NKC):
                    tp = tps.tile([128, 96], fp32, tag="tpw")
                    nc.tensor.matmul(
                        tp,
                        lhsT=wrow_bf2[:, kc * 128 : (kc + 1) * 128],
                        rhs=ident[:96, :96],
                    )
                    nc.scalar.copy(wvT[:, kc, rc * 96 : (rc + 1) * 96], tp)

            for nch in range(S // 512):
                for sc in range(4 * nch, 4 * nch + 4):
                    hrow_bf = stage.tile([128, HID], bf16, tag="hrow_bf")
                    nc.gpsimd.dma_start(
                        out=hrow_bf, in_=hs_d[sc * 128 : (sc + 1) * 128, :]
                    )
                    for kc in range(NKC):
                        tp2 = tps.tile([128, 128], fp32, tag="tpw")
                        nc.tensor.matmul(
                            tp2, lhsT=hrow_bf[:, kc * 128 : (kc + 1) * 128], rhs=ident
                        )
                        dst = hsT[:, kc, sc * 128 : (sc + 1) * 128]
                        if kc % 2 == 0:
                            nc.scalar.copy(dst, tp2)
                        else:
                            nc.vector.tensor_copy(dst, tp2)

                sl = slice(nch * 512, (nch + 1) * 512)
                for h in range(HPC):
                    # q and k together: psum rows 0:64 = qT, 64:128 = kT
                    ps = pps.tile([128, 512], fp32, tag="ps_qk")
                    for kc in range(NKC):
                        nc.tensor.matmul(
                            ps,
                            lhsT=wqkT[:, kc, h * 128 : (h + 1) * 128],
                            rhs=hsT[:, kc, sl],
                            start=(kc == 0),
                            stop=False,
                        )
                    nc.tensor.matmul(
                        ps,
                        lhsT=bias_qk[:, h * 128 : (h + 1) * 128],
                        rhs=ones_row[:, sl],
                        start=False,
                        stop=True,
                    )
                    nc.scalar.copy(qT[h][:, sl], ps[0:HD, :])
                    nc.scalar.copy(kT[h][:, sl], ps[HD:128, :])

                for sc in range(4 * nch, 4 * nch + 4):
                    psv = pps.tile([128, NWID], fp32, tag="ps_v", bufs=2)
                    ssl = slice(sc * 128, (sc + 1) * 128)
                    for kc in range(NKC):
                        nc.tensor.matmul(
                            psv,
                            lhsT=hsT[:, kc, ssl],
                            rhs=wvT[:, kc, :],
                            start=(kc == 0),
                            stop=False,
                        )
                    nc.tensor.matmul(
                        psv, lhsT=ones_row[:, ssl], rhs=bias_v, start=False, stop=True
                    )
                    for h in range(HPC):
                        nc.scalar.copy(
                            v_aug[h][:, sc, 0:HD], psv[:, h * HD : (h + 1) * HD]
                        )

        # ---- Phase 2: attention ----
        with ExitStack() as ph:
            mp = ph.enter_context(tc.tile_pool(name="mp", bufs=4))
            rp = ph.enter_context(tc.tile_pool(name="rp", bufs=10))
            lp = ph.enter_context(tc.tile_pool(name="lp", bufs=5))
            prp = ph.enter_context(tc.tile_pool(name="prp", bufs=5))
            op = ph.enter_context(tc.tile_pool(name="op", bufs=6))
            sps = ph.enter_context(tc.tile_pool(name="sps", bufs=3, space="PSUM"))
            tps2 = ph.enter_context(tc.tile_pool(name="tps2", bufs=2, space="PSUM"))
            cps = ph.enter_context(tc.tile_pool(name="cps", bufs=1, space="PSUM"))

            for si in range(NSC):
                ssl = slice(si * 128, (si + 1) * 128)
                mask_t = mp.tile([128, S], i8, tag="mask")
                nc.sync.dma_start(out=mask_t, in_=mask_d[ssl, :])
                # madd = NEG * mask, fp16, shared across the 3 heads
                madd = mp.tile([128, S], bf16, tag="madd")
                nc.vector.tensor_scalar(
                    out=madd, in0=mask_t, scalar1=NEG, scalar2=None, op0=Alu.mult
                )
                ot = op.tile([128, NWID], fp32, tag="ot")
                for h in range(HPC):
                    # SWDGE cast-DMA: fp32 HBM -> fp16 SBUF, then rel2d
                    # accumulates onto it during its own DMA (CCE add)
                    use_accum = (si * HPC + h) % 8 == 0
                    rel_t = rp.tile([128, S], bf16, tag="rel")
                    nc.gpsimd.dma_start(out=rel_t, in_=rel_d[h, ssl, :])
                    if use_accum:
                        nc.gpsimd.dma_start(
                            out=rel_t, in_=rel2_d[h, ssl, :], accum_op=Alu.add
                        )
                    else:
                        rel2_t = rp.tile([128, S], bf16, tag="rel2", bufs=5)
                        nc.gpsimd.dma_start(out=rel2_t, in_=rel2_d[h, ssl, :])

                    # scores = (rel[+rel2d]) + q k^T accumulated on PE, then
                    # DVE evicts PSUM with fused +madd -> fp16 logits
                    logits = lp.tile([128, S], bf16, tag="logits")
                    for tch in range(S // 512):
                        tsl = slice(tch * 512, (tch + 1) * 512)
                        sc_ps = sps.tile([128, 512], fp32, tag="sc")
                        # q k^T first: it has no DMA dependency, so the PE
                        # can run ahead of the rel loads
                        nc.tensor.matmul(
                            sc_ps, lhsT=qT[h][:, ssl], rhs=kT[h][:, tsl],
                            start=True, stop=False,
                        )
                        if not use_accum:
                            nc.tensor.matmul(
                                sc_ps, lhsT=ident, rhs=rel2_t[:, tsl],
                                start=False, stop=False,
                            )
                        nc.tensor.matmul(
                            sc_ps, lhsT=ident, rhs=rel_t[:, tsl],
                            start=False, stop=True,
                        )
                        nc.vector.tensor_add(logits[:, tsl], sc_ps, madd[:, tsl])

                    # transpose logit blocks as regular matmuls (keeps the
                    # PE HAM-warm); probsT[:, tb*128+j] has rows=t, cols=s
                    probsT = prp.tile([128, S], bf16, tag="probsT")
                    for half in range(2):
                        ltp = tps2.tile([128, 8 * 128], fp32, tag="ltp")
                        for tb in range(8):
                            bsl = slice((half * 8 + tb) * 128, (half * 8 + tb + 1) * 128)
                            nc.tensor.matmul(
                                ltp[:, tb * 128 : (tb + 1) * 128],
                                lhsT=logits[:, bsl],
                                rhs=ident,
                            )
                        nc.scalar.activation(
                            probsT[:, half * 1024 : (half + 1) * 1024],
                            ltp,
                            mybir.ActivationFunctionType.Exp,
                        )

                    ctx_ps = cps.tile([128, HD + 1], fp32, tag="ctx")
                    for tb in range(NSC):
                        bsl = slice(tb * 128, (tb + 1) * 128)
                        nc.tensor.matmul(
                            ctx_ps,
                            lhsT=probsT[:, bsl],
                            rhs=v_aug[h][:, tb, :],
                            start=(tb == 0),
                            stop=(tb == NSC - 1),
                        )

                    rec = op.tile([128, 1], fp32, tag="rec")
                    nc.vector.reciprocal(rec, ctx_ps[:, HD : HD + 1])
                    nc.vector.tensor_scalar(
                        out=ot[:, h * HD : (h + 1) * HD], in0=ctx_ps[:, 0:HD],
                        scalar1=rec, scalar2=None, op0=Alu.mult,
                    )
                nc.sync.dma_start(out=out_d[ssl, :], in_=ot)

    nc.compile()
    return nc


def get_nc():
    if "nc" not in _CACHE:
        _CACHE["nc"] = _build()
    return _CACHE["nc"]


def make_in_maps(
    hidden_states, rel_pos, rel_2d_pos, attention_mask, Wq, bq, Wk, bk, Wv, bv
):
    hidden_states = np.asarray(hidden_states, dtype=np.float32)
    rel_pos = np.asarray(rel_pos, dtype=np.float32)
    rel_2d_pos = np.asarray(rel_2d_pos, dtype=np.float32)
    attention_mask = np.asarray(attention_mask, dtype=np.int8)
    Wq = np.asarray(Wq, dtype=np.float32)
    bq = np.asarray(bq, dtype=np.float32)
    Wk = np.asarray(Wk, dtype=np.float32)
    bk = np.asarray(bk, dtype=np.float32)
    Wv = np.asarray(Wv, dtype=np.float32)
    bv = np.asarray(bv, dtype=np.float32)

    scale = 1.0 / np.sqrt(np.float32(HD))
    in_maps = []
    for c in range(NCORES):
        b = c // 4
        h0 = HPC * (c % 4)
        rsl = slice(HD * h0, HD * (h0 + HPC))
        in_maps.append(
            {
                "hs": hidden_states[b],
                "wq": Wq[rsl] * scale,
                "wk": Wk[rsl],
                "wv": Wv[rsl],
                "bq": (bq[rsl] * scale).reshape(1, -1),
                "bk": bk[rsl].reshape(1, -1),
                "bv": bv[rsl].reshape(1, -1),
                "rel": rel_pos[b, h0 : h0 + HPC],
                "rel2": rel_2d_pos[b, h0 : h0 + HPC],
                "mask": attention_mask[b, 0],
            }
        )
    return in_maps


def gather_out(results):
    out = np.empty((B, S, HID), dtype=np.float32)
    for c in range(NCORES):
        b = c // 4
        g = c % 4
        out[b, :, g * HPC * HD : (g + 1) * HPC * HD] = results[c]["out"]
    return out


def kernel(**inputs) -> np.ndarray:
    from concourse import bass_utils

    nc = get_nc()
    in_maps = make_in_maps(**inputs)
    res = bass_utils.run_bass_kernel_spmd(nc, in_maps, core_ids=list(range(NCORES)))
    return gather_out(res.results)

